# revision 3
# baseline (speedup 1.0000x reference)
"""Trainium2 Bass kernel for nn_Damping — bf16 + diagonal-fold layout.

Math (per sample b):
  h  = tanh MLPs of x0 -> diag xd [64], offdiag z [2016] (strict lower tri L)
  y  = L^T x0 ; D = L y

Layout: feature-major [feature partitions, batch free], batch tile f=512.
All matmuls bf16 (measured ~270ns/512-free slot vs ~520ns fp32r).

Diagonal fold: strict-lower entries are ordered by pairing diagonal c with
diagonal 64-c (lengths 64-c and c -> exactly 64), giving 32 chunks of 64
entries (2016 real + 32 pad). Chunks pack in pairs into [128, f] tiles, so:
  - z production: 16 pair-tiles x 2 K-halves = 32 matmul slots
  - gather (x0g[p] = x0[row(p)]): 0/1 lhsT [64, 128] -> 8 slots/matvec
  - scatter (y_j += u_p): 0/1 lhsT [128, 64] -> 8 accumulating slots/matvec
  - u = z * x0g: 8 DVE tensor_tensor ops/matvec (z bf16 SBUF x pg PSUM)
boo folds into the z PSUM->SBUF drain (ACT bias); bdo into the xd drain;
the diag terms (xd*x0, xd*y) add during the acc drains on DVE. No Q matmuls.

Data parallel over 8 cores: batch 32768 -> 8 x 4096.
"""

import sys

if "/opt/trn_rl_repo" not in sys.path:
    sys.path.insert(0, "/opt/trn_rl_repo")

import numpy as np
import ml_dtypes

N = 64
H = 256
B = 32768
OFF = 2016
OFFP = 2048           # 32 fold chunks x 64
NPAIR = 16            # [128, f] chunk-pair tiles per matvec
NZP = 16              # z pair tiles (OFFP / 128)
NCORES = 8
B_CORE = B // NCORES  # 4096
F = 512               # batch tile (free dim)


def _fold_maps():
    """rows/cols [OFFP] for the diagonal-fold entry order; -1 on pad slots."""
    rows = np.full(OFFP, -1, np.int64)
    cols = np.full(OFFP, -1, np.int64)
    for c in range(1, 33):
        base = (c - 1) * 64
        for q in range(64 - c):          # diag c
            rows[base + q] = q + c
            cols[base + q] = q
        if c != 32:                      # folded diag 64-c
            for q in range(64 - c, 64):
                rows[base + q] = q
                cols[base + q] = q - (64 - c)
    return rows, cols


def _build_nc(b_core=B_CORE, f=F, reps=1):
    """reps>1 unrolls the batch loop inside one NEFF for steady-state timing."""
    import concourse.bacc as bacc
    import concourse.mybir as mybir
    import concourse.tile as tile

    F32 = mybir.dt.float32
    F32R = mybir.dt.float32r
    BF16 = mybir.dt.bfloat16
    Tanh = mybir.ActivationFunctionType.Tanh
    Copy = mybir.ActivationFunctionType.Copy
    Mult = mybir.AluOpType.mult
    Add = mybir.AluOpType.add

    ntiles = b_core // f
    assert b_core % f == 0 and f % 128 == 0
    ncol = f // 128

    nc = bacc.Bacc("TRN2", target_bir_lowering=False, debug=False,
                   num_devices=NCORES)

    # --- DRAM tensors ---
    x_d = nc.dram_tensor("x", [b_core, N], F32R, kind="ExternalInput")
    wd1_d = nc.dram_tensor("wd1t", [N, H], BF16, kind="ExternalInput")
    wo1_d = nc.dram_tensor("wo1t", [N, H], BF16, kind="ExternalInput")
    wd2_d = nc.dram_tensor("wd2t", [H, H], BF16, kind="ExternalInput")
    wo2_d = nc.dram_tensor("wo2t", [H, H], BF16, kind="ExternalInput")
    wdo_d = nc.dram_tensor("wdot", [H, N], BF16, kind="ExternalInput")
    woo_d = nc.dram_tensor("woot", [H, OFFP], BF16, kind="ExternalInput")
    gy_d = nc.dram_tensor("gy", [N, NPAIR * 128], BF16, kind="ExternalInput")
    gd_d = nc.dram_tensor("gd", [N, NPAIR * 128], BF16, kind="ExternalInput")
    sy_d = nc.dram_tensor("sy", [128, NPAIR * N], BF16, kind="ExternalInput")
    sd_d = nc.dram_tensor("sd", [128, NPAIR * N], BF16, kind="ExternalInput")
    id_d = nc.dram_tensor("ident", [128, 128], F32R, kind="ExternalInput")
    bias_d = nc.dram_tensor("biases", [128, 25], F32, kind="ExternalInput")
    out_d = nc.dram_tensor("out", [b_core, N], F32, kind="ExternalOutput")

    with tile.TileContext(nc) as tc:
        with (
            tc.tile_pool(name="wpool", bufs=1) as wp,
            tc.tile_pool(name="apool", bufs=1) as ap,
            tc.tile_pool(name="zpool", bufs=1) as zp,
            tc.tile_pool(name="gpool", bufs=1) as gp,
            tc.tile_pool(name="upool", bufs=1) as up,
            tc.tile_pool(name="iopool", bufs=1) as iop,
            tc.tile_pool(name="psum", bufs=1, space="PSUM") as pp,
        ):
            def wtile(name, src, shape, dt=BF16, q=None):
                t = wp.tile(shape, dt, tag=name, name=name, bufs=1)
                (q or nc.sync).dma_start(t[:], src)
                return t

            # light weights first (unblock tile 0)
            ident = wtile("ident", id_d[:], [128, 128], F32R)
            wd1 = wtile("wd1", wd1_d[:], [N, H])
            wo1 = wtile("wo1", wo1_d[:], [N, H])
            biasT = wtile("biasT", bias_d[:], [128, 25], F32)
            bd1 = [biasT[:, k:k + 1] for k in range(2)]
            bd2 = [biasT[:, 2 + k:3 + k] for k in range(2)]
            bo1 = [biasT[:, 4 + k:5 + k] for k in range(2)]
            bo2 = [biasT[:, 6 + k:7 + k] for k in range(2)]
            boo = [biasT[:, 8 + m:9 + m] for m in range(NZP)]
            bdo = biasT[0:N, 24:25]

            # x-input DMA ring
            total_tiles = ntiles * reps
            x_tiles = {}

            def emit_xdma(tt):
                if tt >= total_tiles:
                    return
                t = tt % ntiles
                xt = iop.tile([128, ncol, N], F32R, tag="x_in", bufs=3,
                              name=f"x_in{tt}")
                nc.sync.dma_start(
                    xt[:], x_d[t * f:(t + 1) * f, :]
                    .rearrange("(c p) n -> p c n", p=128))
                x_tiles[tt] = xt

            emit_xdma(0)
            emit_xdma(1)

            # heavy weights
            wd2 = [wtile(f"wd2_{k}", wd2_d[k * 128:(k + 1) * 128, :], [128, H])
                   for k in range(2)]
            wo2 = [wtile(f"wo2_{k}", wo2_d[k * 128:(k + 1) * 128, :], [128, H])
                   for k in range(2)]
            woo = [wtile(f"woo_{k}", woo_d[k * 128:(k + 1) * 128, :],
                         [128, OFFP]) for k in range(2)]
            # gather/scatter 0/1 matrices: coalesced in DRAM, one DMA each
            gyt = wtile("gyt", gy_d[:], [N, NPAIR * 128], q=nc.gpsimd)
            syt = wtile("syt", sy_d[:], [128, NPAIR * N], q=nc.gpsimd)
            gdt = wtile("gdt", gd_d[:], [N, NPAIR * 128], q=nc.gpsimd)
            sdt = wtile("sdt", sd_d[:], [128, NPAIR * N], q=nc.gpsimd)
            gy = [gyt[:, p * 128:(p + 1) * 128] for p in range(NPAIR)]
            gd = [gdt[:, p * 128:(p + 1) * 128] for p in range(NPAIR)]
            sy = [syt[:, p * N:(p + 1) * N] for p in range(NPAIR)]
            sd = [sdt[:, p * N:(p + 1) * N] for p in range(NPAIR)]
            wdo = [wtile(f"wdo_{k}", wdo_d[k * 128:(k + 1) * 128, :], [128, N],
                         q=nc.gpsimd) for k in range(2)]

            # ---- pipeline stages (generators; yield = PE-group boundary) ----
            def stage_ab(tt, st):
                """Input, transposes, MLPs, z production for tile tt."""
                xt = x_tiles.pop(tt)
                emit_xdma(tt + 2)
                t = f"r{tt}"
                px = pp.tile([N, ncol, 128], F32R, tag="sm", bufs=2,
                             name=f"px{t}")
                for c in range(ncol):
                    nc.tensor.transpose(px[:, c, :], xt[:, c, :], ident[:])
                yield
                x0T = ap.tile([N, f], BF16, tag="x0T", bufs=3, name=f"x0T{t}")
                nc.scalar.activation(x0T[:], px.rearrange("p c n -> p (c n)"),
                                     Copy)
                st["x0T"] = x0T

                def layer(tag, wts, rhss, biases):
                    outs = []
                    for m in range(2):
                        ph = pp.tile([128, f], F32, tag="ph", bufs=3,
                                     name=f"ph_{tag}{m}_{t}")
                        nk = len(wts)
                        for k in range(nk):
                            nc.tensor.matmul(
                                ph[:], wts[k][:, m * 128:(m + 1) * 128],
                                rhss[k][:, :], start=(k == 0),
                                stop=(k == nk - 1))
                        h = ap.tile([128, f], BF16, tag=f"{tag}{m}", bufs=3,
                                    name=f"{tag}{m}_{t}")
                        nc.scalar.activation(h[:], ph[:], Tanh,
                                             bias=biases[m])
                        outs.append(h)
                        yield
                    return outs

                h1o = yield from layer("h1o", [wo1], [x0T], bo1)
                h1d = yield from layer("h1d", [wd1], [x0T], bd1)
                h2o = yield from layer("h2o", wo2, h1o, bo2)
                h2d = yield from layer("h2d", wd2, h1d, bd2)

                # z pair tiles (longest pole), diag head after
                z_sb = []
                for m in range(NZP):
                    pz = pp.tile([128, f], F32, tag="ph", bufs=3,
                                 name=f"pz{m}_{t}")
                    for k in range(2):
                        nc.tensor.matmul(
                            pz[:], woo[k][:, m * 128:(m + 1) * 128], h2o[k][:],
                            start=(k == 0), stop=(k == 1))
                    zt = zp.tile([128, f], BF16, tag=f"z{m}", bufs=3,
                                 name=f"z{m}_{t}")
                    # drain + fold the boo bias in (Identity activation)
                    nc.scalar.add(zt[:], pz[:], boo[m])
                    z_sb.append(zt)
                    yield
                st["z"] = z_sb

                pxd = pp.tile([128, f], F32, tag="ph", bufs=3, name=f"pxd{t}")
                for k in range(2):
                    nc.tensor.matmul(pxd[0:N, :], wdo[k][:], h2d[k][:],
                                     start=(k == 0), stop=(k == 1))
                xd = ap.tile([N, f], BF16, tag="xd", bufs=2, name=f"xd{t}")
                nc.scalar.add(xd[:], pxd[0:N, :], bdo)
                st["xd"] = xd
                t1 = ap.tile([N, f], BF16, tag="t1", bufs=2, name=f"t1_{t}")
                nc.vector.tensor_tensor(t1[:], xd[:], x0T[:], Mult)
                st["t1"] = t1
                yield

            def matvec(t, rhs, z_sb, gmats, smats, acc, name):
                """acc[0:N] = sum_p smats[p]^T (z_p * (gmats[p]^T rhs)).
                Software-pipelined: gather p+1 issues before scatter p."""
                pgs = {}
                us = {}

                def emit_gather(p):
                    if p >= NPAIR:
                        return
                    pg = pp.tile([128, f], F32, tag="pg", bufs=3,
                                 name=f"pg_{name}{p}_{t}")
                    nc.tensor.matmul(pg[:], gmats[p], rhs[:],
                                     start=True, stop=True)
                    pgs[p] = pg

                def emit_umul(p):
                    u = up.tile([128, f], BF16, tag="u", bufs=6,
                                name=f"u_{name}{p}_{t}")
                    nc.vector.tensor_tensor(u[:], z_sb[p][:], pgs.pop(p)[:],
                                            Mult)
                    us[p] = u

                emit_gather(0)
                emit_gather(1)
                emit_gather(2)
                emit_umul(0)
                for p in range(NPAIR):
                    emit_gather(p + 3)
                    if p + 1 < NPAIR:
                        emit_umul(p + 1)
                    nc.tensor.matmul(acc[0:N, :], smats[p], us.pop(p)[:],
                                     start=(p == 0), stop=(p == NPAIR - 1),
                                     skip_group_check=(p > 0))
                    yield

            def stage_cd(tt, st):
                """Both L matvecs + output for tile tt."""
                t_out = tt % ntiles
                t = f"r{tt}"
                x0T, z_sb, xd, t1 = st["x0T"], st["z"], st["xd"], st["t1"]

                acc_y = pp.tile([128, f], F32, tag="sm", bufs=2,
                                name=f"py{t}")
                yield from matvec(t, x0T, z_sb, gy, sy, acc_y, "y")
                y = ap.tile([N, f], BF16, tag="y", bufs=2, name=f"y{t}")
                nc.vector.tensor_tensor(y[:], acc_y[0:N, :], t1[:], Add)
                t2 = ap.tile([N, f], BF16, tag="t2", bufs=2, name=f"t2_{t}")
                nc.vector.tensor_tensor(t2[:], xd[:], y[:], Mult)
                yield

                acc_d = pp.tile([128, f], F32, tag="sm", bufs=2,
                                name=f"pd{t}")
                yield from matvec(t, y, z_sb, gd, sd, acc_d, "d")
                dd = ap.tile([N, f], F32R, tag="dd", bufs=2, name=f"dd{t}")
                nc.vector.tensor_tensor(dd[:], acc_d[0:N, :], t2[:], Add)
                yield

                po = pp.tile([128, ncol, N], F32R, tag="sm", bufs=2,
                             name=f"po{t}")
                for c in range(ncol):
                    nc.tensor.transpose(po[:, c, :],
                                        dd[:, c * 128:(c + 1) * 128],
                                        ident[:N, :N])
                o_sb = iop.tile([128, ncol, N], F32, tag="o_sb", bufs=2,
                                name=f"o_sb{t}")
                nc.scalar.activation(o_sb[:], po.rearrange("p c n -> p (c n)"),
                                     Copy)
                nc.gpsimd.dma_start(
                    out_d[t_out * f:(t_out + 1) * f, :]
                    .rearrange("(c p) n -> p c n", p=128), o_sb[:])
                yield

            # ---- driver: round-robin CD(t) with AB(t+1) ----
            def drain(g):
                for _ in g:
                    pass

            states = {0: {}}
            drain(stage_ab(0, states[0]))
            for tt in range(total_tiles):
                gens = [stage_cd(tt, states[tt])]
                if tt + 1 < total_tiles:
                    states[tt + 1] = {}
                    gens.append(stage_ab(tt + 1, states[tt + 1]))
                while gens:
                    for g in list(gens):
                        try:
                            next(g)
                        except StopIteration:
                            gens.remove(g)
                del states[tt]

    nc.compile()
    return nc


def _host_constants(Wd1, bd1, Wd2, bd2, Wdo, bdo, Wo1, bo1, Wo2, bo2, Woo, boo):
    """Shared (per-core replicated) input arrays."""
    f32 = np.float32
    bf16 = ml_dtypes.bfloat16
    rows, cols = _fold_maps()
    valid = rows >= 0

    # map original tri order -> fold slots
    tri_r, tri_c = np.tril_indices(N, k=-1)
    tri_pos = {(i, j): k for k, (i, j) in enumerate(zip(tri_r, tri_c))}
    fold_src = np.array([tri_pos[(rows[p], cols[p])] if valid[p] else -1
                         for p in range(OFFP)])

    Woo_f = np.zeros((OFFP, H), f32)
    Woo_f[valid] = np.asarray(Woo, f32)[fold_src[valid]]
    boo_f = np.zeros(OFFP, f32)
    boo_f[valid] = np.asarray(boo, f32)[fold_src[valid]]

    gy = np.zeros((NPAIR, N, 128), f32)
    gd = np.zeros((NPAIR, N, 128), f32)
    sy = np.zeros((NPAIR, 128, N), f32)
    sd = np.zeros((NPAIR, 128, N), f32)
    for p in range(NPAIR):
        for m in range(128):
            s = 128 * p + m
            if rows[s] < 0:
                continue
            gy[p, rows[s], m] = 1.0
            gd[p, cols[s], m] = 1.0
            sy[p, m, cols[s]] = 1.0
            sd[p, m, rows[s]] = 1.0

    def bt(a):
        return np.ascontiguousarray(np.asarray(a, f32).astype(bf16))

    def ct(a):
        return np.ascontiguousarray(a, dtype=f32)

    return {
        "wd1t": bt(np.asarray(Wd1).T), "wd2t": bt(np.asarray(Wd2).T),
        "wdot": bt(np.asarray(Wdo).T), "wo1t": bt(np.asarray(Wo1).T),
        "wo2t": bt(np.asarray(Wo2).T), "woot": bt(Woo_f.T),
        "gy": bt(gy.transpose(1, 0, 2).reshape(N, NPAIR * 128)),
        "gd": bt(gd.transpose(1, 0, 2).reshape(N, NPAIR * 128)),
        "sy": bt(sy.transpose(1, 0, 2).reshape(128, NPAIR * N)),
        "sd": bt(sd.transpose(1, 0, 2).reshape(128, NPAIR * N)),
        "ident": np.eye(128, dtype=f32),
        "biases": ct(np.concatenate([
            np.asarray(bd1, f32).reshape(2, 128).T,
            np.asarray(bd2, f32).reshape(2, 128).T,
            np.asarray(bo1, f32).reshape(2, 128).T,
            np.asarray(bo2, f32).reshape(2, 128).T,
            boo_f.reshape(NZP, 128).T,
            np.pad(np.asarray(bdo, f32).reshape(1, N),
                   ((0, 0), (0, 128 - N))).T,
        ], axis=1)),
    }


_NC_CACHE = {}


def get_nc(b_core=B_CORE, f=F, reps=1):
    key = (b_core, f, reps)
    if key not in _NC_CACHE:
        _NC_CACHE[key] = _build_nc(b_core, f, reps)
    return _NC_CACHE[key]


def make_in_maps(input, **params):
    shared = _host_constants(**params)
    x = np.ascontiguousarray(np.asarray(input), dtype=np.float32)
    assert x.shape == (B, N)
    return [dict(shared, x=x[c * B_CORE:(c + 1) * B_CORE]) for c in range(NCORES)]


def kernel(input, **params):
    from concourse import bass_utils

    nc = get_nc()
    in_maps = make_in_maps(input, **params)
    res = bass_utils.run_bass_kernel_spmd(nc, in_maps,
                                          core_ids=list(range(NCORES)))
    return np.concatenate([r["out"] for r in res.results], axis=0)


# revision 4
# speedup vs baseline: 1.0293x; 1.0293x over previous
"""Trainium2 Bass kernel for nn_Damping — bf16 + diagonal-fold layout.

Math (per sample b):
  h  = tanh MLPs of x0 -> diag xd [64], offdiag z [2016] (strict lower tri L)
  y  = L^T x0 ; D = L y

Layout: feature-major [feature partitions, batch free], batch tile f=512.
All matmuls bf16 (measured ~270ns/512-free slot vs ~520ns fp32r).

Diagonal fold: strict-lower entries are ordered by pairing diagonal c with
diagonal 64-c (lengths 64-c and c -> exactly 64), giving 32 chunks of 64
entries (2016 real + 32 pad). Chunks pack in pairs into [128, f] tiles, so:
  - z production: 16 pair-tiles x 2 K-halves = 32 matmul slots
  - gather (x0g[p] = x0[row(p)]): 0/1 lhsT [64, 128] -> 8 slots/matvec
  - scatter (y_j += u_p): 0/1 lhsT [128, 64] -> 8 accumulating slots/matvec
  - u = z * x0g: 8 DVE tensor_tensor ops/matvec (z bf16 SBUF x pg PSUM)
boo folds into the z PSUM->SBUF drain (ACT bias); bdo into the xd drain;
the diag terms (xd*x0, xd*y) add during the acc drains on DVE. No Q matmuls.

Data parallel over 8 cores: batch 32768 -> 8 x 4096.
"""

import sys

if "/opt/trn_rl_repo" not in sys.path:
    sys.path.insert(0, "/opt/trn_rl_repo")

import numpy as np
import ml_dtypes

N = 64
H = 256
B = 32768
OFF = 2016
OFFP = 2048           # 32 fold chunks x 64
NPAIR = 16            # [128, f] chunk-pair tiles per matvec
NZP = 16              # z pair tiles (OFFP / 128)
NCORES = 8
B_CORE = B // NCORES  # 4096
F = 512               # batch tile (free dim)


def _fold_maps():
    """rows/cols [OFFP] for the diagonal-fold entry order; -1 on pad slots."""
    rows = np.full(OFFP, -1, np.int64)
    cols = np.full(OFFP, -1, np.int64)
    for c in range(1, 33):
        base = (c - 1) * 64
        for q in range(64 - c):          # diag c
            rows[base + q] = q + c
            cols[base + q] = q
        if c != 32:                      # folded diag 64-c
            for q in range(64 - c, 64):
                rows[base + q] = q
                cols[base + q] = q - (64 - c)
    return rows, cols


def _build_nc(b_core=B_CORE, f=F, reps=1):
    """reps>1 unrolls the batch loop inside one NEFF for steady-state timing."""
    import concourse.bacc as bacc
    import concourse.mybir as mybir
    import concourse.tile as tile

    F32 = mybir.dt.float32
    F32R = mybir.dt.float32r
    BF16 = mybir.dt.bfloat16
    Tanh = mybir.ActivationFunctionType.Tanh
    Copy = mybir.ActivationFunctionType.Copy
    Mult = mybir.AluOpType.mult
    Add = mybir.AluOpType.add

    ntiles = b_core // f
    assert b_core % f == 0 and f % 128 == 0
    ncol = f // 128

    nc = bacc.Bacc("TRN2", target_bir_lowering=False, debug=False,
                   num_devices=NCORES)

    # --- DRAM tensors ---
    x_d = nc.dram_tensor("x", [N, b_core], BF16, kind="ExternalInput")
    wd1_d = nc.dram_tensor("wd1t", [N, H], BF16, kind="ExternalInput")
    wo1_d = nc.dram_tensor("wo1t", [N, H], BF16, kind="ExternalInput")
    wd2_d = nc.dram_tensor("wd2t", [H, H], BF16, kind="ExternalInput")
    wo2_d = nc.dram_tensor("wo2t", [H, H], BF16, kind="ExternalInput")
    wdo_d = nc.dram_tensor("wdot", [H, N], BF16, kind="ExternalInput")
    woo_d = nc.dram_tensor("woot", [H, OFFP], BF16, kind="ExternalInput")
    gy_d = nc.dram_tensor("gy", [N, NPAIR * 128], BF16, kind="ExternalInput")
    gd_d = nc.dram_tensor("gd", [N, NPAIR * 128], BF16, kind="ExternalInput")
    sy_d = nc.dram_tensor("sy", [128, NPAIR * N], BF16, kind="ExternalInput")
    sd_d = nc.dram_tensor("sd", [128, NPAIR * N], BF16, kind="ExternalInput")
    bias_d = nc.dram_tensor("biases", [128, 25], F32, kind="ExternalInput")
    out_d = nc.dram_tensor("out", [N, b_core], F32, kind="ExternalOutput")

    with tile.TileContext(nc) as tc:
        with (
            tc.tile_pool(name="wpool", bufs=1) as wp,
            tc.tile_pool(name="apool", bufs=1) as ap,
            tc.tile_pool(name="zpool", bufs=1) as zp,
            tc.tile_pool(name="gpool", bufs=1) as gp,
            tc.tile_pool(name="upool", bufs=1) as up,
            tc.tile_pool(name="iopool", bufs=1) as iop,
            tc.tile_pool(name="psum", bufs=1, space="PSUM") as pp,
        ):
            def wtile(name, src, shape, dt=BF16, q=None):
                t = wp.tile(shape, dt, tag=name, name=name, bufs=1)
                (q or nc.sync).dma_start(t[:], src)
                return t

            # light weights first (unblock tile 0)
            wd1 = wtile("wd1", wd1_d[:], [N, H])
            wo1 = wtile("wo1", wo1_d[:], [N, H])
            biasT = wtile("biasT", bias_d[:], [128, 25], F32)
            bd1 = [biasT[:, k:k + 1] for k in range(2)]
            bd2 = [biasT[:, 2 + k:3 + k] for k in range(2)]
            bo1 = [biasT[:, 4 + k:5 + k] for k in range(2)]
            bo2 = [biasT[:, 6 + k:7 + k] for k in range(2)]
            boo = [biasT[:, 8 + m:9 + m] for m in range(NZP)]
            bdo = biasT[0:N, 24:25]

            # x-input DMA ring
            total_tiles = ntiles * reps
            x_tiles = {}

            def emit_xdma(tt):
                if tt >= total_tiles:
                    return
                t = tt % ntiles
                xt = iop.tile([N, f], BF16, tag="x_in", bufs=3,
                              name=f"x_in{tt}")
                nc.sync.dma_start(xt[:], x_d[:, t * f:(t + 1) * f])
                x_tiles[tt] = xt

            emit_xdma(0)
            emit_xdma(1)

            # heavy weights
            wd2 = [wtile(f"wd2_{k}", wd2_d[k * 128:(k + 1) * 128, :], [128, H])
                   for k in range(2)]
            wo2 = [wtile(f"wo2_{k}", wo2_d[k * 128:(k + 1) * 128, :], [128, H])
                   for k in range(2)]
            woo = [wtile(f"woo_{k}", woo_d[k * 128:(k + 1) * 128, :],
                         [128, OFFP]) for k in range(2)]
            # gather/scatter 0/1 matrices: coalesced in DRAM, one DMA each
            gyt = wtile("gyt", gy_d[:], [N, NPAIR * 128], q=nc.gpsimd)
            syt = wtile("syt", sy_d[:], [128, NPAIR * N], q=nc.gpsimd)
            gdt = wtile("gdt", gd_d[:], [N, NPAIR * 128], q=nc.gpsimd)
            sdt = wtile("sdt", sd_d[:], [128, NPAIR * N], q=nc.gpsimd)
            gy = [gyt[:, p * 128:(p + 1) * 128] for p in range(NPAIR)]
            gd = [gdt[:, p * 128:(p + 1) * 128] for p in range(NPAIR)]
            sy = [syt[:, p * N:(p + 1) * N] for p in range(NPAIR)]
            sd = [sdt[:, p * N:(p + 1) * N] for p in range(NPAIR)]
            wdo = [wtile(f"wdo_{k}", wdo_d[k * 128:(k + 1) * 128, :], [128, N],
                         q=nc.gpsimd) for k in range(2)]

            # ---- pipeline stages (generators; yield = PE-group boundary) ----
            def stage_ab(tt, st):
                """Input, transposes, MLPs, z production for tile tt."""
                x0T = x_tiles.pop(tt)
                emit_xdma(tt + 2)
                t = f"r{tt}"
                st["x0T"] = x0T

                def layer(tag, wts, rhss, biases):
                    outs = []
                    for m in range(2):
                        ph = pp.tile([128, f], F32, tag="ph", bufs=3,
                                     name=f"ph_{tag}{m}_{t}")
                        nk = len(wts)
                        for k in range(nk):
                            nc.tensor.matmul(
                                ph[:], wts[k][:, m * 128:(m + 1) * 128],
                                rhss[k][:, :], start=(k == 0),
                                stop=(k == nk - 1))
                        h = ap.tile([128, f], BF16, tag=f"{tag}{m}", bufs=3,
                                    name=f"{tag}{m}_{t}")
                        nc.scalar.activation(h[:], ph[:], Tanh,
                                             bias=biases[m])
                        outs.append(h)
                        yield
                    return outs

                h1o = yield from layer("h1o", [wo1], [x0T], bo1)
                h1d = yield from layer("h1d", [wd1], [x0T], bd1)
                h2o = yield from layer("h2o", wo2, h1o, bo2)
                h2d = yield from layer("h2d", wd2, h1d, bd2)

                # z pair tiles (longest pole), diag head after
                z_sb = []
                for m in range(NZP):
                    pz = pp.tile([128, f], F32, tag="ph", bufs=3,
                                 name=f"pz{m}_{t}")
                    for k in range(2):
                        nc.tensor.matmul(
                            pz[:], woo[k][:, m * 128:(m + 1) * 128], h2o[k][:],
                            start=(k == 0), stop=(k == 1))
                    zt = zp.tile([128, f], BF16, tag=f"z{m}", bufs=3,
                                 name=f"z{m}_{t}")
                    # drain + fold the boo bias in (Identity activation)
                    nc.scalar.add(zt[:], pz[:], boo[m])
                    z_sb.append(zt)
                    yield
                st["z"] = z_sb

                pxd = pp.tile([128, f], F32, tag="ph", bufs=3, name=f"pxd{t}")
                for k in range(2):
                    nc.tensor.matmul(pxd[0:N, :], wdo[k][:], h2d[k][:],
                                     start=(k == 0), stop=(k == 1))
                xd = ap.tile([N, f], BF16, tag="xd", bufs=2, name=f"xd{t}")
                nc.scalar.add(xd[:], pxd[0:N, :], bdo)
                st["xd"] = xd
                t1 = ap.tile([N, f], BF16, tag="t1", bufs=2, name=f"t1_{t}")
                nc.vector.tensor_tensor(t1[:], xd[:], x0T[:], Mult)
                st["t1"] = t1
                yield

            def matvec(t, rhs, z_sb, gmats, smats, acc, name):
                """acc[0:N] = sum_p smats[p]^T (z_p * (gmats[p]^T rhs)).
                Software-pipelined: gather p+1 issues before scatter p."""
                pgs = {}
                us = {}

                def emit_gather(p):
                    if p >= NPAIR:
                        return
                    pg = pp.tile([128, f], F32, tag="pg", bufs=3,
                                 name=f"pg_{name}{p}_{t}")
                    nc.tensor.matmul(pg[:], gmats[p], rhs[:],
                                     start=True, stop=True)
                    pgs[p] = pg

                def emit_umul(p):
                    u = up.tile([128, f], BF16, tag="u", bufs=6,
                                name=f"u_{name}{p}_{t}")
                    nc.vector.tensor_tensor(u[:], z_sb[p][:], pgs.pop(p)[:],
                                            Mult)
                    us[p] = u

                emit_gather(0)
                emit_gather(1)
                emit_gather(2)
                emit_umul(0)
                for p in range(NPAIR):
                    emit_gather(p + 3)
                    if p + 1 < NPAIR:
                        emit_umul(p + 1)
                    nc.tensor.matmul(acc[0:N, :], smats[p], us.pop(p)[:],
                                     start=(p == 0), stop=(p == NPAIR - 1),
                                     skip_group_check=(p > 0))
                    yield

            def stage_cd(tt, st):
                """Both L matvecs + output for tile tt."""
                t_out = tt % ntiles
                t = f"r{tt}"
                x0T, z_sb, xd, t1 = st["x0T"], st["z"], st["xd"], st["t1"]

                acc_y = pp.tile([128, f], F32, tag="sm", bufs=2,
                                name=f"py{t}")
                yield from matvec(t, x0T, z_sb, gy, sy, acc_y, "y")
                y = ap.tile([N, f], BF16, tag="y", bufs=2, name=f"y{t}")
                nc.vector.tensor_tensor(y[:], acc_y[0:N, :], t1[:], Add)
                t2 = ap.tile([N, f], BF16, tag="t2", bufs=2, name=f"t2_{t}")
                nc.vector.tensor_tensor(t2[:], xd[:], y[:], Mult)
                yield

                acc_d = pp.tile([128, f], F32, tag="sm", bufs=2,
                                name=f"pd{t}")
                yield from matvec(t, y, z_sb, gd, sd, acc_d, "d")
                dd = ap.tile([N, f], F32, tag="dd", bufs=2, name=f"dd{t}")
                nc.vector.tensor_tensor(dd[:], acc_d[0:N, :], t2[:], Add)
                nc.gpsimd.dma_start(out_d[:, t_out * f:(t_out + 1) * f],
                                    dd[:])
                yield

            # ---- driver: round-robin CD(t) with AB(t+1) ----
            def drain(g):
                for _ in g:
                    pass

            states = {0: {}}
            drain(stage_ab(0, states[0]))
            for tt in range(total_tiles):
                gens = [stage_cd(tt, states[tt])]
                if tt + 1 < total_tiles:
                    states[tt + 1] = {}
                    gens.append(stage_ab(tt + 1, states[tt + 1]))
                while gens:
                    for g in list(gens):
                        try:
                            next(g)
                        except StopIteration:
                            gens.remove(g)
                del states[tt]

    nc.compile()
    return nc


def _host_constants(Wd1, bd1, Wd2, bd2, Wdo, bdo, Wo1, bo1, Wo2, bo2, Woo, boo):
    """Shared (per-core replicated) input arrays."""
    f32 = np.float32
    bf16 = ml_dtypes.bfloat16
    rows, cols = _fold_maps()
    valid = rows >= 0

    # map original tri order -> fold slots
    tri_r, tri_c = np.tril_indices(N, k=-1)
    tri_pos = {(i, j): k for k, (i, j) in enumerate(zip(tri_r, tri_c))}
    fold_src = np.array([tri_pos[(rows[p], cols[p])] if valid[p] else -1
                         for p in range(OFFP)])

    Woo_f = np.zeros((OFFP, H), f32)
    Woo_f[valid] = np.asarray(Woo, f32)[fold_src[valid]]
    boo_f = np.zeros(OFFP, f32)
    boo_f[valid] = np.asarray(boo, f32)[fold_src[valid]]

    gy = np.zeros((NPAIR, N, 128), f32)
    gd = np.zeros((NPAIR, N, 128), f32)
    sy = np.zeros((NPAIR, 128, N), f32)
    sd = np.zeros((NPAIR, 128, N), f32)
    for p in range(NPAIR):
        for m in range(128):
            s = 128 * p + m
            if rows[s] < 0:
                continue
            gy[p, rows[s], m] = 1.0
            gd[p, cols[s], m] = 1.0
            sy[p, m, cols[s]] = 1.0
            sd[p, m, rows[s]] = 1.0

    def bt(a):
        return np.ascontiguousarray(np.asarray(a, f32).astype(bf16))

    def ct(a):
        return np.ascontiguousarray(a, dtype=f32)

    return {
        "wd1t": bt(np.asarray(Wd1).T), "wd2t": bt(np.asarray(Wd2).T),
        "wdot": bt(np.asarray(Wdo).T), "wo1t": bt(np.asarray(Wo1).T),
        "wo2t": bt(np.asarray(Wo2).T), "woot": bt(Woo_f.T),
        "gy": bt(gy.transpose(1, 0, 2).reshape(N, NPAIR * 128)),
        "gd": bt(gd.transpose(1, 0, 2).reshape(N, NPAIR * 128)),
        "sy": bt(sy.transpose(1, 0, 2).reshape(128, NPAIR * N)),
        "sd": bt(sd.transpose(1, 0, 2).reshape(128, NPAIR * N)),
        "biases": ct(np.concatenate([
            np.asarray(bd1, f32).reshape(2, 128).T,
            np.asarray(bd2, f32).reshape(2, 128).T,
            np.asarray(bo1, f32).reshape(2, 128).T,
            np.asarray(bo2, f32).reshape(2, 128).T,
            boo_f.reshape(NZP, 128).T,
            np.pad(np.asarray(bdo, f32).reshape(1, N),
                   ((0, 0), (0, 128 - N))).T,
        ], axis=1)),
    }


_NC_CACHE = {}


def get_nc(b_core=B_CORE, f=F, reps=1):
    key = (b_core, f, reps)
    if key not in _NC_CACHE:
        _NC_CACHE[key] = _build_nc(b_core, f, reps)
    return _NC_CACHE[key]


def make_in_maps(input, **params):
    shared = _host_constants(**params)
    x = np.asarray(input, np.float32).astype(ml_dtypes.bfloat16)
    assert x.shape == (B, N)
    return [dict(shared,
                 x=np.ascontiguousarray(x[c * B_CORE:(c + 1) * B_CORE].T))
            for c in range(NCORES)]


def kernel(input, **params):
    from concourse import bass_utils

    nc = get_nc()
    in_maps = make_in_maps(input, **params)
    res = bass_utils.run_bass_kernel_spmd(nc, in_maps,
                                          core_ids=list(range(NCORES)))
    return np.concatenate([np.ascontiguousarray(r["out"].T)
                           for r in res.results], axis=0)
